# revision 3
# baseline (speedup 1.0000x reference)
"""Dense GAT (2-layer, 8+1 heads) on 8 Trainium2 NeuronCores — V3.

Row-parallel over destination rows i (R=512/core), dense over sources j.

Per head, the unnormalized attention
    e[j,i] = adj[i,j] * max(eas[j]*ead[i], e2as[j]*e2ad[i])
is divided by the per-i factor ead[i] (cancels in the softmax):
    e'[j,i] = adj[i,j] * max(eas[j], e2as[j]*r[i]),  r = exp(-0.8*a_dst)
so each tile is ONE 4x-mode tensor_scalar (two per-partition f32 scalars)
plus a mask tensor_tensor shared across a head-pair (stride-0 broadcast of
adjT), split between DVE and GPSIMD. Heads 6,7 use the ACT chain
exp(prelu(a_src+a_dst)) instead (per-head consistent scaling).
Aggregation runs transposed: out^T[i,c] via 65-col matmuls with e-tiles as
lhsT; softmax division is a per-partition tensor_scalar. adjT arrives
pre-transposed from the host. One AllGather moves h2 (+ layer-2 scalars).

PSUM rule: matmul start=True zeroes the whole bank, so only the first
chain emitted per bank carries start=True; sibling slots accumulate with
start=False onto the zeroed bank.
"""
import numpy as np

N = 4096
F_IN = 256
HID = 64
H1 = 8
F1 = H1 * HID
OUT = 128
N_CORES = 8
R = N // N_CORES
JT = N // 128
IT = R // 128
NEG_ATT = 0.2
NEG_OUT = 0.01
NB = 2          # heads 6,7 on the ACT chain
G2 = OUT + 8    # fp8 gather cols: h2[128] | 3x f16-bitcast scalars | pad

_CACHE = {}


def _gp_mask_grps(jt):
    # head-pair groups whose mask TT runs on GPSIMD for this jt;
    # taper the last jts so GPSIMD doesn't drain after DVE/ACT finish
    if jt >= 28:
        return (1,) if jt % 2 else ()
    return (1, 2) if jt % 2 else (1,)


def _build():
    import concourse.bass as bass
    from concourse import bacc
    import concourse.mybir as mybir
    import concourse.tile as tile
    from concourse.masks import make_identity

    f32 = mybir.dt.float32
    f16 = mybir.dt.float16
    f8 = mybir.dt.float8e4
    A = mybir.ActivationFunctionType
    Al = mybir.AluOpType

    nc = bacc.Bacc("TRN2", target_bir_lowering=False, debug=False,
                   num_devices=N_CORES)
    d_xT = nc.dram_tensor("xT", [F_IN, N], f16, kind="ExternalInput")
    d_xmT = nc.dram_tensor("xmT", [F_IN, R], f16, kind="ExternalInput")
    d_adjT = nc.dram_tensor("adjT", [N, R], f16, kind="ExternalInput")
    d_rhs1 = nc.dram_tensor("rhs1", [F_IN, F1], f16, kind="ExternalInput")
    d_vsrc1 = nc.dram_tensor("vsrc1", [F_IN, H1], f16, kind="ExternalInput")
    d_vdst1 = nc.dram_tensor("vdst1", [F_IN, H1], f16, kind="ExternalInput")
    d_rhs2 = nc.dram_tensor("rhs2", [F1, OUT + 2], f16, kind="ExternalInput")
    d_b1c = nc.dram_tensor("b1c", [HID, H1], f32, kind="ExternalInput")
    d_b2r = nc.dram_tensor("b2r", [1, OUT], f32, kind="ExternalInput")
    d_out = nc.dram_tensor("out", [R, OUT], f32, kind="ExternalOutput")

    with tile.TileContext(nc) as tc:
        with tc.tile_pool(name="const", bufs=1) as const, \
             tc.tile_pool(name="big", bufs=1) as big, \
             tc.tile_pool(name="dram", bufs=1, space="DRAM") as dram:
            ident16 = const.tile([128, 128], f16)
            make_identity(nc, ident16)
            ones_col16 = const.tile([128, 1], f16)
            nc.vector.memset(ones_col16, 1.0)

            rhs1_sb = const.tile([128, 2, F1], f16)
            vsrc1_sb = const.tile([128, 2, H1], f16)
            vdst1_sb = const.tile([128, 2, H1], f16)
            xmT_sb = const.tile([128, 2, R], f16)
            for kb in range(2):
                ks = slice(kb * 128, (kb + 1) * 128)
                nc.sync.dma_start(out=rhs1_sb[:, kb, :], in_=d_rhs1[ks, :])
                nc.sync.dma_start(out=vsrc1_sb[:, kb, :], in_=d_vsrc1[ks, :])
                nc.sync.dma_start(out=vdst1_sb[:, kb, :], in_=d_vdst1[ks, :])
                nc.sync.dma_start(out=xmT_sb[:, kb, :], in_=d_xmT[ks, :])
            rhs2_sb = const.tile([128, 4, OUT + 2], f16)
            b1c_sb = const.tile([HID, H1], f32)
            b2b_sb = const.tile([128, OUT], f32)

            xT_sb = big.tile([128, 2, N], f16)
            for kb in range(2):
                nc.sync.dma_start(out=xT_sb[:, kb, 0:1024],
                                  in_=d_xT[kb * 128:kb * 128 + 128, 0:1024])
            for kb in range(2):
                nc.sync.dma_start(out=xT_sb[:, kb, 1024:N],
                                  in_=d_xT[kb * 128:kb * 128 + 128, 1024:N])
            # adjV: [j=128, jt, i=512] fp16 {32768,0}, direct strided DMA.
            # jh0 lands before the phase-1 staging DMAs; jh1-3 queue after.
            adjT_all = big.tile([128, JT, R], f16)

            def load_adjv(jh):
                nc.sync.dma_start(
                    out=adjT_all[:, jh * 8:(jh + 1) * 8, :],
                    in_=d_adjT[jh * 1024:(jh + 1) * 1024, :].rearrange(
                        "(jt p) i -> p jt i", p=128))

            load_adjv(0)

            h1_all = big.tile([128, JT, H1, HID + 1], f16)
            nc.vector.memset(h1_all[:, :, :, HID:HID + 1], 1.0)
            asrc_all = big.tile([128, JT, H1], f32)
            eas_all = big.tile([128, JT, H1], f32)
            e2as_all = big.tile([128, JT, H1], f32)
            adstT = big.tile([H1, R], f32)
            r16row = big.tile([H1, R], f16)
            ad16row = big.tile([H1, R], f16)
            rb_all = big.tile([128, H1 - NB, R], f16)
            adB_all = big.tile([128, NB, R], f16)
            x2T_all = big.tile([128, 4, R], f16)
            h2g_all = big.tile([128, JT, G2], f8)
            sc2_all = big.tile([128, JT, 2], f32)
            ad2row = big.tile([1, R], f32)

            stg_r = dram.tile([H1, R], f16, name="stg_r")
            stg_ad = dram.tile([H1, R], f16, name="stg_ad")
            stg_r2 = dram.tile([1, R], f16, name="stg_r2")

            # ---------------- phase 1: a_dst, r, broadcasts ----------------
            with tc.tile_pool(name="p1w", bufs=2) as p1w, \
                 tc.tile_pool(name="p1ps", bufs=2, space="PSUM") as p1ps:
                for it in range(IT):
                    ps_ad = p1ps.tile([128, H1], f32, tag="ad")
                    for kb in range(2):
                        nc.tensor.matmul(
                            ps_ad, xmT_sb[:, kb, it * 128:(it + 1) * 128],
                            vdst1_sb[:, kb, :],
                            start=(kb == 0), stop=(kb == 1))
                    adm = p1w.tile([128, H1], f16, tag="adm")
                    nc.vector.tensor_copy(adm, ps_ad)
                    ps_adT = p1ps.tile([H1, 128], f16, tag="adT")
                    nc.tensor.transpose(ps_adT, adm, ident16)
                    nc.vector.tensor_copy(adstT[:, it * 128:(it + 1) * 128],
                                          ps_adT)
                nc.scalar.activation(r16row, adstT, A.Exp,
                                     scale=-(1 - NEG_ATT))
                nc.vector.tensor_copy(ad16row, adstT)
                nc.sync.dma_start(out=stg_r, in_=r16row)
                nc.sync.dma_start(out=stg_ad, in_=ad16row)
                for h in range(H1 - NB):
                    nc.sync.dma_start(
                        out=rb_all[:, h, :],
                        in_=stg_r[h:h + 1, :].broadcast_to((128, R)))
                for hb in range(NB):
                    hs = H1 - NB + hb
                    nc.sync.dma_start(
                        out=adB_all[:, hb, :],
                        in_=stg_ad[hs:hs + 1, :].broadcast_to((128, R)))

            for jh in range(1, 4):
                load_adjv(jh)
            nc.sync.dma_start(out=b1c_sb, in_=d_b1c[:, :])

            # ---------------- phase 2+3: h1 (interleaved) + E + agg --------
            NGRP = H1 // 2
            with tc.tile_pool(name="ew", bufs=2) as ew, \
                 tc.tile_pool(name="post", bufs=2) as post, \
                 tc.tile_pool(name="ps_h", bufs=2, space="PSUM") as ps_hp, \
                 tc.tile_pool(name="ps_a", bufs=1, space="PSUM") as ps_ap, \
                 tc.tile_pool(name="ps_g", bufs=1, space="PSUM") as ps_gp:
                # full-bank accumulators: agg_a[g] slots 0..6, agg_s slot 7
                # (one per group). slot = hh*4+ib.
                agg_a = [ps_gp.tile([128, 512], f32, name=f"agA{g}")
                         for g in range(NGRP)]
                agg_s = ps_gp.tile([128, 512], f32, name="agS")
                ps_asrc = ps_ap.tile([128, 2, H1], f32)

                def agg_slot(g, hh, ib):
                    s = hh * 4 + ib
                    if s < 7:
                        return agg_a[g][:, s * 65:(s + 1) * 65]
                    return agg_s[:, g * 65:(g + 1) * 65]

                def agg_start(g, hh, ib, jt):
                    if jt != 0:
                        return False
                    s = hh * 4 + ib
                    return s == 0 or (s == 7 and g == 1)

                def emit_h1(jt):
                    ps_h = ps_hp.tile([128, F1], f32, tag="h1")
                    cols = slice(jt * 128, (jt + 1) * 128)
                    for kb in range(2):
                        nc.tensor.matmul(ps_h, xT_sb[:, kb, cols],
                                         rhs1_sb[:, kb, :],
                                         start=(kb == 0), stop=(kb == 1))
                    for kb in range(2):
                        nc.tensor.matmul(ps_asrc[:, jt % 2, :],
                                         xT_sb[:, kb, cols],
                                         vsrc1_sb[:, kb, :],
                                         start=(kb == 0), stop=(kb == 1))
                    dst = h1_all[:, jt, :, 0:HID]
                    src = ps_h.rearrange("p (h c) -> p h c", c=HID)
                    if jt % 2 == 0:
                        nc.vector.tensor_copy(dst, src)
                    else:
                        nc.scalar.copy(dst, src)
                    nc.scalar.copy(asrc_all[:, jt, :], ps_asrc[:, jt % 2, :])
                    if jt % 4 == 3:
                        gsl = slice(jt - 3, jt + 1)
                        nc.scalar.activation(eas_all[:, gsl, :],
                                             asrc_all[:, gsl, :], A.Exp)
                        nc.scalar.activation(e2as_all[:, gsl, :],
                                             asrc_all[:, gsl, :], A.Exp,
                                             scale=NEG_ATT)

                def post_group(g):
                    ps_y = ps_hp.tile([HID, R], f16, tag="h1",
                                      name=f"psy{g}")
                    for hh in range(2):
                        h = g * 2 + hh
                        y16 = post.tile([128, 4, HID], f16, tag="y16",
                                        name=f"y{g}{hh}")
                        for ib in range(4):
                            sl = agg_slot(g, hh, ib)
                            rz = post.tile([128, 1], f32, tag="rz", bufs=4)
                            nc.vector.reciprocal(rz, sl[:, HID:HID + 1])
                            nc.vector.tensor_scalar_mul(
                                y16[:, ib, :], sl[:, 0:HID], rz)
                        for ib in range(4):
                            nc.tensor.transpose(
                                ps_y[:, ib * 128:(ib + 1) * 128],
                                y16[:, ib, :], ident16)
                        po = (h % 2) * HID
                        nc.scalar.activation(
                            x2T_all[po:po + HID, h // 2, :], ps_y, A.Prelu,
                            bias=b1c_sb[:, h:h + 1], alpha=NEG_OUT)

                def emit_b_produce(jt):
                    # ACT chain for heads 6,7 (group 3), one jt ahead
                    t2 = ew.tile([128, 2, R], f16, tag="t3", bufs=3,
                                 name=f"t3_{jt % 3}")
                    for hb in range(NB):
                        h = H1 - NB + hb
                        t1 = ew.tile([128, R], f32, tag=f"bt{hb}", bufs=2)
                        nc.scalar.activation(
                            t1, adB_all[:, hb, :], A.Prelu,
                            bias=asrc_all[:, jt, h:h + 1], alpha=NEG_ATT)
                        nc.scalar.activation(t2[:, hb, :], t1, A.Exp)
                    return t2

                def emit_mask_agg(g, t2, jt, gp):
                    adj_b = adjT_all[:, jt:jt + 1, :].broadcast_to(
                        (128, 2, R))
                    e2 = ew.tile([128, 2, R], f16, tag=f"e{g}", bufs=3,
                                 name=f"e{g}_{jt % 3}")
                    if gp:
                        nc.gpsimd.tensor_tensor(out=e2, in0=t2,
                                                in1=adj_b, op=Al.mult)
                    else:
                        nc.vector.tensor_tensor(out=e2, in0=t2,
                                                in1=adj_b, op=Al.mult)
                    for hh in range(2):
                        h = g * 2 + hh
                        for ib in range(4):
                            nc.tensor.matmul(
                                agg_slot(g, hh, ib),
                                e2[:, hh, ib * 128:(ib + 1) * 128],
                                h1_all[:, jt, h, :],
                                start=agg_start(g, hh, ib, jt),
                                stop=(jt == JT - 1))
                    if jt == JT - 1:
                        post_group(g)

                for jp in range(8):
                    emit_h1(jp)
                t2b = emit_b_produce(0)
                for jt in range(JT):
                    if jt + 8 < JT:
                        emit_h1(jt + 8)
                    for g in (1, 0, 2):
                        t2 = ew.tile([128, 2, R], f16, tag=f"t{g}", bufs=3,
                                     name=f"t{g}_{jt % 3}")
                        for hh in range(2):
                            h = g * 2 + hh
                            nc.vector.tensor_scalar(
                                t2[:, hh, :], rb_all[:, h, :],
                                e2as_all[:, jt, h:h + 1],
                                eas_all[:, jt, h:h + 1],
                                op0=Al.mult, op1=Al.max)
                        emit_mask_agg(g, t2, jt, g in _gp_mask_grps(jt))
                    t2b_cur = t2b
                    if jt + 1 < JT:
                        t2b = emit_b_produce(jt + 1)
                    emit_mask_agg(3, t2b_cur, jt, False)

            # ---------------- phase 4: h2 + gather ----------------
            for kt in range(4):
                nc.sync.dma_start(out=rhs2_sb[:, kt, :],
                                  in_=d_rhs2[kt * 128:(kt + 1) * 128, :])
            nc.sync.dma_start(out=b2b_sb,
                              in_=d_b2r[0:1, :].broadcast_to((128, OUT)))
            bounce_in = dram.tile([R, G2], f8, name="bin")
            bounce_out = dram.tile([N_CORES, R, G2], f8,
                                   addr_space="Shared", name="bout")
            with tc.tile_pool(name="l2w", bufs=2) as l2w, \
                 tc.tile_pool(name="ps_l2", bufs=1, space="PSUM") as ps_l2:
                ps_adt2 = ps_l2.tile([1, 4, 128], f16)
                ps_h2s = [ps_l2.tile([128, OUT + 2], f32, name=f"ph2_{ib}")
                          for ib in range(4)]
                for kt in range(4):
                    for ib in range(4):
                        nc.tensor.matmul(
                            ps_h2s[ib],
                            x2T_all[:, kt, ib * 128:(ib + 1) * 128],
                            rhs2_sb[:, kt, :],
                            start=(kt == 0), stop=(kt == 3))
                for ib in range(4):
                    ps_h2 = ps_h2s[ib]
                    h2m = l2w.tile([128, G2], f8, tag="h2m")
                    nc.vector.tensor_copy(h2m[:, 0:OUT], ps_h2[:, 0:OUT])
                    h2s = h2m[:, OUT:OUT + 6].bitcast(f16)
                    nc.vector.tensor_copy(h2s[:, 0:1], ps_h2[:, OUT:OUT + 1])
                    nc.scalar.activation(h2s[:, 1:2],
                                         ps_h2[:, OUT:OUT + 1], A.Exp)
                    nc.scalar.activation(h2s[:, 2:3],
                                         ps_h2[:, OUT:OUT + 1], A.Exp,
                                         scale=NEG_ATT)
                    a2d16 = l2w.tile([128, 1], f16, tag="a2d")
                    nc.vector.tensor_copy(a2d16, ps_h2[:, OUT + 1:OUT + 2])
                    nc.tensor.transpose(ps_adt2[0:1, ib, :], a2d16, ident16)
                    nc.sync.dma_start(
                        out=bounce_in[ib * 128:(ib + 1) * 128, :], in_=h2m)
                nc.vector.tensor_copy(
                    ad2row, ps_adt2[0:1, :, :].rearrange("a b c -> a (b c)"))
                r2row = big.tile([1, R], f16)
                nc.scalar.activation(r2row, ad2row, A.Exp,
                                     scale=-(1 - NEG_ATT))
                nc.sync.dma_start(out=stg_r2, in_=r2row)
                rb2 = big.tile([128, R], f16)
                nc.sync.dma_start(out=rb2,
                                  in_=stg_r2[0:1, :].broadcast_to((128, R)))
                nc.gpsimd.collective_compute(
                    "AllGather", bass.mybir.AluOpType.bypass,
                    replica_groups=[list(range(N_CORES))],
                    ins=[bounce_in.opt()],
                    outs=[bounce_out.opt()],
                )
                nc.sync.dma_start(
                    out=h2g_all,
                    in_=bounce_out.rearrange("c (rl p) g -> p (c rl) g",
                                             p=128))

            # ---------------- phase 5: layer-2 attention ----------------
            with tc.tile_pool(name="fw", bufs=2) as fw, \
                 tc.tile_pool(name="ps_f", bufs=1, space="PSUM") as ps_f:
                nc.vector.tensor_copy(
                    sc2_all,
                    h2g_all[:, :, OUT:OUT + 6].bitcast(f16)[:, :, 1:3])
                agg2 = [ps_f.tile([128, 512], f32, name=f"ag2_{ib}")
                        for ib in range(4)]
                for jp in range(JT // 2):
                    t = fw.tile([128, 2, R], f16, tag="t2l", bufs=6)
                    for u in range(2):
                        jt = jp * 2 + u
                        nc.vector.tensor_scalar(
                            t[:, u, :], rb2, sc2_all[:, jt, 1:2],
                            sc2_all[:, jt, 0:1], op0=Al.mult, op1=Al.max)
                    e = fw.tile([128, 2, R], f16, tag="e2l", bufs=6)
                    if jp % 3 == 2:
                        nc.gpsimd.tensor_tensor(
                            out=e, in0=t,
                            in1=adjT_all[:, jp * 2:jp * 2 + 2, :], op=Al.mult)
                    else:
                        nc.vector.tensor_tensor(
                            out=e, in0=t,
                            in1=adjT_all[:, jp * 2:jp * 2 + 2, :], op=Al.mult)
                    for u in range(2):
                        jt = jp * 2 + u
                        for ib in range(4):
                            nc.tensor.matmul(
                                agg2[ib][:, 0:OUT],
                                e[:, u, ib * 128:(ib + 1) * 128],
                                h2g_all[:, jt, 0:OUT],
                                start=(jt == 0), stop=(jt == JT - 1))
                            nc.tensor.matmul(
                                agg2[ib][:, OUT:OUT + 1],
                                e[:, u, ib * 128:(ib + 1) * 128], ones_col16,
                                start=False, stop=(jt == JT - 1))
                for ib in range(4):
                    rz2 = fw.tile([128, 1], f32, tag="rz2", bufs=2)
                    nc.vector.reciprocal(rz2, agg2[ib][:, OUT:OUT + 1])
                    y2 = fw.tile([128, OUT], f32, tag="y2", bufs=2)
                    nc.vector.tensor_scalar_mul(y2, agg2[ib][:, 0:OUT], rz2)
                    yb = fw.tile([128, OUT], f32, tag="yb", bufs=2)
                    nc.vector.tensor_add(yb, y2, b2b_sb)
                    of = fw.tile([128, OUT], f32, tag="of", bufs=2)
                    nc.scalar.activation(of, yb, A.Prelu, alpha=NEG_OUT)
                    nc.sync.dma_start(out=d_out[ib * 128:(ib + 1) * 128, :],
                                      in_=of)

    nc.finalize()
    return nc


def _prep_host(x, adj, w1, att_src1, att_dst1, b1, w2, att_src2, att_dst2, b2):
    x = np.asarray(x, np.float32).reshape(N, F_IN)
    adj = np.asarray(adj, np.float32).reshape(N, N)
    w1 = np.asarray(w1, np.float32)
    w2 = np.asarray(w2, np.float32)
    att_src1 = np.asarray(att_src1, np.float32)
    att_dst1 = np.asarray(att_dst1, np.float32)
    att_src2 = np.asarray(att_src2, np.float32)
    att_dst2 = np.asarray(att_dst2, np.float32)
    b1 = np.asarray(b1, np.float32)
    b2 = np.asarray(b2, np.float32)

    xT16 = np.ascontiguousarray(x.T).astype(np.float16)
    adj16 = adj.astype(np.float16)
    v_src1 = np.empty((F_IN, H1), np.float32)
    v_dst1 = np.empty((F_IN, H1), np.float32)
    for h in range(H1):
        blk = w1[:, h * HID:(h + 1) * HID]
        v_src1[:, h] = blk @ att_src1[h]
        v_dst1[:, h] = blk @ att_dst1[h]
    v_src2 = (w2 @ att_src2[0])[:, None]
    v_dst2 = (w2 @ att_dst2[0])[:, None]
    rhs2 = np.concatenate([w2, v_src2, v_dst2], axis=1).astype(np.float16)
    b1c = np.ascontiguousarray(b1.reshape(H1, HID).T)
    b2r = np.ascontiguousarray(b2.reshape(1, OUT))
    w1_16 = w1.astype(np.float16)
    v_src1_16 = v_src1.astype(np.float16)
    v_dst1_16 = v_dst1.astype(np.float16)

    in_maps = []
    for c in range(N_CORES):
        rows = slice(c * R, (c + 1) * R)
        in_maps.append({
            "xT": xT16,
            "xmT": np.ascontiguousarray(xT16[:, rows]),
            "adjT": np.ascontiguousarray(adj16[rows, :].T),
            "rhs1": w1_16,
            "vsrc1": v_src1_16,
            "vdst1": v_dst1_16,
            "rhs2": rhs2,
            "b1c": b1c,
            "b2r": b2r,
        })
    return in_maps


def kernel(**inputs) -> np.ndarray:
    from concourse.bass_utils import run_bass_kernel_spmd

    if "nc" not in _CACHE:
        _CACHE["nc"] = _build()
    nc = _CACHE["nc"]
    in_maps = _prep_host(**inputs)
    try:
        res = run_bass_kernel_spmd(nc, in_maps, list(range(N_CORES)))
    except Exception:
        res = run_bass_kernel_spmd(nc, in_maps, list(range(N_CORES)))
    out = np.empty((1, N, OUT), np.float32)
    for c in range(N_CORES):
        out[0, c * R:(c + 1) * R, :] = res.results[c]["out"]
    return out
